# revision 29
# baseline (speedup 1.0000x reference)
"""Trainium2 Bass kernel for DampedAttention.

Full inputs in, full output out. Sharding: 8 cores = 2 batches x 4 head-groups
(4 heads of dim 64 each per core). Per core:

  proj   V first (dc-outer so matmuls chase the input DMA; bias opens each
         PSUM accumulation), then QT/KT (qb-outer; bias fused into the ACT
         psum->sbuf copy as a per-partition bias AP). Scale 1/8 folded into
         Wq/bq on host.
  attn   per (q-block, head-pair, k-chunk): scores S^T = K^T_chunk . Q
         (row-packed head pairs via tile_position), softmax exp split across
         two engines: ACT computes exact Exp -> fp8e4, DVE computes the
         Schraudolph bit trick (int8 = s*8/ln2 + 55.656 == fp8e4 bits of
         e^s) -- both write planes of a persistent fp8 ring [128,16,2,512].
         P@V runs in fp8e4 DoubleRow (2 k-chunks per instruction, 2x PE
         throughput); a ones column in V8 gives softmax row-sums for free.
  blend  1/rowsum on DVE reciprocal (keeps ACT exp-table resident), gpsimd
         partition_broadcast, DVE mul + scalar_tensor_tensor into ctxt bf16.
  LV     banded 0.4*L^T matmuls in bf16 (fp8 would distort the dominant
         local term), column-packed head pairs.
  out    out-projection interleaved per q-block right after its blend;
         psum->sbuf copies split ACT/DVE, DMAs issued from gpsimd.

Accumulation, row-sums, reciprocal and the blend stay fp32. The entropy
gate in the reference is a forward no-op and is skipped. Softmax
max-subtraction is skipped (scores are O(1), no overflow; fp8 exp values
stay in [e^-6, e^6] well inside e4m3 range).
"""
import numpy as np
import ml_dtypes

S = 2048
D = 1024
CLOC = 256          # channels per core (4 heads x 64)
HD = 64
NH = 4              # heads per core
NDC = 8             # 128-wide d-chunks in contraction D
NKC = 16            # 128-wide k/s chunks in S
NKP = 8             # k-chunk pairs (fp8 DoubleRow granularity)
NQB = 4             # 512-wide q blocks
QB = 512
WINDOW = 3
STRENGTH = 0.4
EPS = 1e-10
F32 = np.float32
BF16 = ml_dtypes.bfloat16
FP8 = ml_dtypes.float8_e4m3fn

# k-chunks whose softmax exp runs on DVE (Schraudolph bit trick) instead of
# ACT; balance so ACT-exp ~= DVE-exp + DVE blend work.
DVE_KCS = frozenset((1, 4, 7, 10, 12, 15))
EXP_SCALE = float(8.0 / np.log(2.0))     # fp8e4 bits of e^s = s*8/ln2 + B
EXP_BIAS = 55.65625
# Q/K projections run in fp8e4 DoubleRow; weights are pre-scaled by 32 so
# e4m3 sees ~N(0,0.64) values (no subnormals). The combined 32*32/8 = 2^13
# score scale is divided back out inside the exp input scale.
QK_SCALE = 32.0
SC_DESCALE = float(1.0 / (QK_SCALE * QK_SCALE * 8.0))


def _build_L04T():
    i = np.arange(S)
    d = (i[:, None] - i[None, :]).astype(F32)
    k = np.where(np.abs(d) <= WINDOW,
                 np.exp(-(d ** 2) / F32(2.0 * STRENGTH ** 2)),
                 F32(0.0)).astype(F32)
    L = k / (k.sum(axis=-1, keepdims=True) + F32(EPS))
    return (F32(0.4) * L).T.copy()  # [s, q], pre-scaled by (1 - lambda_jump)


def _lt_tiles():
    """Unique [128, 512] band tiles of 0.4*L^T plus (qb -> [(j, uniq_idx)])."""
    L04T = _build_L04T()
    uniq = []
    slots = {qb: [] for qb in range(NQB)}
    for qb in range(NQB):
        for j in range(max(0, qb * 4 - 1), min(NKC, qb * 4 + 5)):
            t = L04T[j * 128:(j + 1) * 128, qb * QB:(qb + 1) * QB]
            for ui, ut in enumerate(uniq):
                if np.array_equal(t, ut):
                    slots[qb].append((j, ui))
                    break
            else:
                slots[qb].append((j, len(uniq)))
                uniq.append(t)
    return np.stack(uniq).astype(BF16), slots


_LT_UNIQ, _LT_SLOTS = _lt_tiles()
NU = _LT_UNIQ.shape[0]

_CACHE = {}


def _w8(w):
    """[CLOC, D] weight slice -> fp8 DoubleRow layout [128, 4, 2, CLOC]."""
    wt = (w.T * F32(QK_SCALE)).astype(FP8)          # [D, CLOC]
    return np.ascontiguousarray(
        wt.reshape(NDC // 2, 2, 128, CLOC).transpose(2, 0, 1, 3))


def _build_program():
    import concourse.bacc as bacc
    import concourse.mybir as mybir
    from concourse.tile import TileContext

    f32 = mybir.dt.float32
    bf16 = mybir.dt.bfloat16
    fp8 = mybir.dt.float8e4
    i8 = mybir.dt.int8
    Exp = mybir.ActivationFunctionType.Exp
    Copy = mybir.ActivationFunctionType.Copy
    Ident = mybir.ActivationFunctionType.Identity
    DR = mybir.MatmulPerfMode.DoubleRow
    mult = mybir.AluOpType.mult
    add = mybir.AluOpType.add

    nc = bacc.Bacc("TRN2", target_bir_lowering=False, debug=False,
                   enable_asserts=False, num_devices=8)

    xt = nc.dram_tensor("xt", [D, S], bf16, kind="ExternalInput").ap()
    wq8d = nc.dram_tensor("wq8d", [128, NDC // 2, 2, CLOC], fp8,
                          kind="ExternalInput").ap()
    wk8d = nc.dram_tensor("wk8d", [128, NDC // 2, 2, CLOC], fp8,
                          kind="ExternalInput").ap()
    wvt = nc.dram_tensor("wvt", [D, CLOC], bf16, kind="ExternalInput").ap()
    bqc = nc.dram_tensor("bqc", [128, 2], f32, kind="ExternalInput").ap()
    bkc = nc.dram_tensor("bkc", [128, 2], f32, kind="ExternalInput").ap()
    bvr = nc.dram_tensor("bvr", [1, CLOC], bf16, kind="ExternalInput").ap()
    wot = nc.dram_tensor("wot", [CLOC, D], bf16, kind="ExternalInput").ap()
    ltt = nc.dram_tensor("ltt", [NU, 128, QB], bf16, kind="ExternalInput").ap()
    out = nc.dram_tensor("out", [S, D], f32, kind="ExternalOutput").ap()

    with TileContext(nc) as tc:
        with tc.tile_pool(name="persist", bufs=1) as pp:
            # ---- persistent SBUF ----
            qt = [pp.tile([128, S], bf16, name=f"qt{i}") for i in range(2)]
            kt = [pp.tile([128, S], bf16, name=f"kt{i}") for i in range(2)]
            v_all = pp.tile([128, NKC, NH, HD], bf16)      # bf16 V for LV
            v8 = pp.tile([128, NKC, NH, 128], fp8)         # fp8 V; ones col 64
            ring = pp.tile([128, 2, NKC, 2, QB], i8)       # exp fp8 bits, x2 parity
            ctxt_all = pp.tile([128, 2, S], bf16)
            wot_sb = pp.tile([128, 2, D], bf16)
            lt_sb = pp.tile([128, NU, QB], bf16)
            bq_sb = pp.tile([128, 2], f32)
            bk_sb = pp.tile([128, 2], f32)
            bv_sb = pp.tile([1, 2, CLOC], bf16)  # bv twice: bank-wide bias row
            ones_c = pp.tile([1, 128], bf16)     # ones row (V bias outer prod)

            # bias loads first (V-proj bias matmuls gate the whole phase);
            # the slow strided v8 ones-column memset goes to idle DVE.
            nc.gpsimd.dma_start(bv_sb[:, 0, :], bvr[:])
            nc.gpsimd.dma_start(bv_sb[:, 1, :], bvr[:])
            nc.gpsimd.dma_start(bq_sb[:], bqc[:])
            nc.gpsimd.dma_start(bk_sb[:], bkc[:])
            nc.gpsimd.memset(ones_c[:], 1.0)
            nc.vector.memset(v8[:, :, :, HD:HD + 1], 1.0)

            # ---- phase B+C: projections then attention ----
            with tc.tile_pool(name="projsb", bufs=1) as prs:
                xt_sb = prs.tile([128, NDC, S], bf16)
                x8_sb = prs.tile([128, NDC // 2, 2, S], fp8)
                wq_sb = prs.tile([128, NDC // 2, 2, CLOC], fp8)
                wk_sb = prs.tile([128, NDC // 2, 2, CLOC], fp8)
                wv_sb = prs.tile([128, NDC, CLOC], bf16)
                nc.gpsimd.dma_start(wq_sb[:], wq8d[:])
                nc.gpsimd.dma_start(wk_sb[:], wk8d[:])
                for dc in range(NDC):
                    dsl = slice(dc * 128, (dc + 1) * 128)
                    nc.sync.dma_start(xt_sb[:, dc, 0:S // 2], xt[dsl, 0:S // 2])
                    nc.sync.dma_start(xt_sb[:, dc, S // 2:S],
                                      xt[dsl, S // 2:S])
                    nc.scalar.dma_start(wv_sb[:, dc, :], wvt[dsl, :])
                # x -> fp8 casts on ACT; only need the xt chunks, so they run
                # while the V-proj psum accumulations chase the same DMAs
                for dc in range(NDC):
                    nc.scalar.activation(x8_sb[:, dc // 2, dc % 2, :],
                                         xt_sb[:, dc, :], Copy)
                # deferred loads (needed only from the attention phase on)
                for cc in range(2):
                    nc.gpsimd.dma_start(wot_sb[:, cc, :],
                                        wot[cc * 128:(cc + 1) * 128, :])
                for u in range(NU):
                    nc.gpsimd.dma_start(lt_sb[:, u, :], ltt[u, :, :])

                # V natural [s-chunk 128, 256]: dc-outer so matmuls chase
                # the input DMA. Wave A covers sc 0-11 in 6 banks (two
                # s-chunks per bank; dc0/slice0 opens the bank, the bias
                # outer-product closes it bank-wide); wave B (sc 12-15)
                # reuses two tags afterwards. QT/KT runs in its own 2-bank
                # pool concurrently with wave B, so attention starts as soon
                # as the pools swap.
                with (
                    tc.tile_pool(name="vps", bufs=1, space="PSUM") as vpp,
                    tc.tile_pool(name="qktps", bufs=2, space="PSUM") as qkp,
                ):
                    vtiles = {}

                    def emit_vwave(scs):
                        for i, sc in enumerate(scs):
                            if sc % 2 == 0:
                                vtiles[sc // 2] = vpp.tile(
                                    [128, 2, CLOC], f32,
                                    tag=f"vps{(sc // 2) % 6}",
                                    name=f"vps{sc // 2}")

                        def vp(sc):
                            return vtiles[sc // 2][:, sc % 2, :]

                        for dc in range(NDC):
                            for sc in scs:
                                nc.tensor.matmul(
                                    vp(sc),
                                    xt_sb[:, dc, sc * 128:(sc + 1) * 128],
                                    wv_sb[:, dc, :],
                                    start=(dc == 0 and sc % 2 == 0),
                                    stop=False, skip_group_check=True)
                        for sc in scs:
                            if sc % 2 == 0:
                                nc.tensor.matmul(
                                    vtiles[sc // 2][:], ones_c[:],
                                    bv_sb[:].rearrange("p a c -> p (a c)"),
                                    start=False, stop=True,
                                    skip_group_check=True)
                        for sc in scs:
                            ps4 = vp(sc).rearrange("p (h e) -> p h e", h=NH)
                            nc.vector.tensor_copy(v_all[:, sc, :, :], ps4)
                            nc.scalar.activation(v8[:, sc, :, 0:HD], ps4, Copy)

                    emit_vwave(list(range(12)))

                    def emit_qkt(ct):
                        csl = slice(ct * 128, (ct + 1) * 128)
                        for dst, w_sb, b_sb in ((qt[ct], wq_sb, bq_sb),
                                                (kt[ct], wk_sb, bk_sb)):
                            for qb in range(NQB):
                                qsl = slice(qb * QB, (qb + 1) * QB)
                                ps = qkp.tile([128, QB], f32, tag="qktps")
                                for dp in range(NDC // 2):
                                    nc.tensor.matmul(
                                        ps[:], w_sb[:, dp, :, csl],
                                        x8_sb[:, dp, :, qsl],
                                        start=(dp == 0),
                                        stop=(dp == NDC // 2 - 1),
                                        perf_mode=DR)
                                nc.scalar.activation(dst[:, qsl], ps[:],
                                                     Ident,
                                                     bias=b_sb[:, ct:ct + 1])

                    emit_qkt(0)
                    emit_vwave(list(range(12, 16)))
                    emit_qkt(1)

                # QT/KT + attention share one PSUM layout: stps 3x2 banks
                # (scores / lv / QKT-proj chains) + ctxps 2x1 banks (PV
                # accumulators / out-proj) = 8 banks. The ct1 proj chains
                # interleave with the first attention iteration.
                attn_pools = (
                    tc.tile_pool(name="stps", bufs=3, space="PSUM"),
                    tc.tile_pool(name="ctxps", bufs=2, space="PSUM"),
                    tc.tile_pool(name="stage", bufs=2),
                    tc.tile_pool(name="osb", bufs=4),
                )
                stp = attn_pools[0].__enter__()
                ctp = attn_pools[1].__enter__()
                sp = attn_pools[2].__enter__()
                osb = attn_pools[3].__enter__()

                def emit_pv(it, ctx, kp):
                    hp = it % 2
                    for hh in range(2):
                        nc.tensor.matmul(
                            ctx[hh][0:HD + 1, :],
                            v8[:, 2 * kp:2 * kp + 2, 2 * hp + hh, 0:HD + 1],
                            ring[:, it % 2, 2 * kp:2 * kp + 2, hh, :]
                            .bitcast(fp8),
                            start=(kp == 0), stop=(kp == NKP - 1),
                            perf_mode=DR)

                def emit_blend(it, ctx):
                    """LV + softmax normalize + blend into ctxt (PVs done)."""
                    qb, hp = it // 2, it % 2
                    qsl = slice(qb * QB, (qb + 1) * QB)
                    lv_ps = stp.tile([128, QB], f32, tag="stps")
                    slots = _LT_SLOTS[qb]
                    for n, (j, u) in enumerate(slots):
                        for hh in range(2):
                            nc.tensor.matmul(
                                lv_ps[hh * HD:(hh + 1) * HD, :],
                                v_all[:, j, 2 * hp + hh, :],
                                lt_sb[:, u, :],
                                start=(n == 0), stop=(n == len(slots) - 1),
                                tile_position=(0, hh * HD),
                                skip_group_check=True)
                    for hh in range(2):
                        rsum = sp.tile([1, QB], f32, tag="rsum")
                        nc.scalar.copy(rsum[:], ctx[hh][64:65, :])
                        rinv = sp.tile([1, QB], f32, tag="rinv")
                        nc.vector.reciprocal_approx_fast(rinv[:], rsum[:])
                        bc = sp.tile([64, QB], f32, tag="bc")
                        nc.gpsimd.partition_broadcast(bc[:], rinv[:],
                                                      channels=HD)
                        m1 = sp.tile([64, QB], f32, tag="m1")
                        nc.vector.tensor_mul(m1[:], ctx[hh][0:HD, :], bc[:])
                        nc.vector.scalar_tensor_tensor(
                            ctxt_all[hh * 64:hh * 64 + 64, hp, qsl],
                            m1[:], 0.6,
                            lv_ps[hh * HD:(hh + 1) * HD, :],
                            op0=mult, op1=add)

                def alloc_ctx():
                    return [ctp.tile([128, QB], f32, tag="ctxps",
                                     name=f"ctx{hh}") for hh in range(2)]

                def emit_scores(it, weave_prev, self_ctx=None):
                    """Scores+exps for `it`. The previous iteration's PV
                    matmuls are woven into the FIRST half of the score
                    stream (kp at kc=kp) and its LV+blend is emitted right
                    after the last PV, so ctx slots free mid-iteration and
                    the PE stays dense (keeps the DVFS ramp at full clock)."""
                    qb, hp = it // 2, it % 2
                    qsl = slice(qb * QB, (qb + 1) * QB)
                    par = it % 2
                    ctx_prev = None
                    for kc in range(NKC):
                        st = stp.tile([128, 2, QB], f32, tag="stps")
                        for hh in range(2):
                            p0 = hh * 64
                            nc.tensor.matmul(
                                st[:, hh, :],
                                kt[hp][p0:p0 + 64, kc * 128:(kc + 1) * 128],
                                qt[hp][p0:p0 + 64, qsl],
                                start=True, stop=True,
                                tile_position=(p0, 0))
                        dst8 = ring[:, par, kc, :, :]
                        if kc in DVE_KCS:
                            nc.vector.tensor_scalar(
                                dst8, st[:], EXP_SCALE * SC_DESCALE, EXP_BIAS,
                                mult, add)
                        else:
                            nc.scalar.activation(dst8.bitcast(fp8), st[:], Exp,
                                                 scale=SC_DESCALE)
                        if weave_prev and kc < NKP:
                            if kc == 0:
                                ctx_prev = alloc_ctx()
                            emit_pv(it - 1, ctx_prev, kc)
                        if weave_prev and kc == NKP:
                            emit_blend(it - 1, ctx_prev)
                        if self_ctx is not None and kc >= NKP + 1:
                            emit_pv(it, self_ctx, kc - NKP - 1)

                def emit_outproj(qb, drain=False):
                    for si in range(4):
                        sc = qb * 4 + si
                        ssl = slice(sc * 128, (sc + 1) * 128)
                        for ot in range(2):
                            if drain:
                                ps = stp.tile([128, QB], f32, tag="stps")
                            else:
                                ps = ctp.tile([128, QB], f32, tag="ctxps")
                            for cc in range(2):
                                nc.tensor.matmul(
                                    ps[:], ctxt_all[:, cc, ssl],
                                    wot_sb[:, cc, ot * QB:(ot + 1) * QB],
                                    start=(cc == 0), stop=(cc == 1))
                            ot_sb = osb.tile([128, QB], f32, tag="osb")
                            if si * 2 + ot < 2:
                                nc.scalar.activation(ot_sb[:], ps[:], Copy)
                            else:
                                nc.vector.tensor_copy(ot_sb[:], ps[:])
                            if drain:
                                hb = QB // 2
                                o0 = ot * QB
                                nc.sync.dma_start(
                                    out[ssl, o0:o0 + hb], ot_sb[:, 0:hb])
                                nc.sync.dma_start(
                                    out[ssl, o0 + hb:o0 + QB], ot_sb[:, hb:QB])
                            else:
                                nc.sync.dma_start(
                                    out[ssl, ot * QB:(ot + 1) * QB], ot_sb[:])

                ctx7 = None
                for it in range(8):
                    if it == 7:
                        ctx7 = alloc_ctx()
                    emit_scores(it, weave_prev=(it > 0), self_ctx=ctx7)
                    if it >= 3 and it % 2 == 1:
                        emit_outproj((it - 3) // 2)
                # drain: last PV + blend of the final iteration, its q-block
                emit_pv(7, ctx7, NKP - 1)
                emit_blend(7, ctx7)
                emit_outproj(3)
                for p in reversed(attn_pools):
                    p.__exit__(None, None, None)

    nc.compile()
    return nc


def _get_program():
    if "nc" not in _CACHE:
        _CACHE["nc"] = _build_program()
    return _CACHE["nc"]


def _in_maps(x, Wq, bq, Wk, bk, Wv, bv, Wo):
    xT = [np.ascontiguousarray(x[b].T).astype(BF16) for b in range(2)]
    maps = []
    for c in range(8):
        b, hg = c // 4, c % 4
        hs, he = hg * CLOC, (hg + 1) * CLOC
        maps.append({
            "xt": xT[b],
            "wq8d": _w8(Wq[hs:he]),
            "wk8d": _w8(Wk[hs:he]),
            "wvt": np.ascontiguousarray(Wv[hs:he].T).astype(BF16),
            "bqc": np.ascontiguousarray(
                (bq[hs:he] * F32(QK_SCALE)).reshape(2, 128).T).astype(F32),
            "bkc": np.ascontiguousarray(
                (bk[hs:he] * F32(QK_SCALE)).reshape(2, 128).T).astype(F32),
            "bvr": bv[hs:he][None, :].astype(BF16),
            "wot": np.ascontiguousarray(Wo[:, hs:he].T).astype(BF16),
            "ltt": _LT_UNIQ,
        })
    return maps


def _run(x, Wq, bq, Wk, bk, Wv, bv, Wo, bo, trace=False):
    from concourse.bass_utils import run_bass_kernel_spmd
    nc = _get_program()
    maps = _in_maps(np.asarray(x, F32), np.asarray(Wq, F32), np.asarray(bq, F32),
                    np.asarray(Wk, F32), np.asarray(bk, F32), np.asarray(Wv, F32),
                    np.asarray(bv, F32), np.asarray(Wo, F32))
    res = run_bass_kernel_spmd(nc, maps, list(range(8)), trace=trace)
    bo = np.asarray(bo, F32)
    outp = np.empty((2, S, D), F32)
    for b in range(2):
        acc = res.results[b * 4]["out"].astype(F32)
        for hg in range(1, 4):
            acc = acc + res.results[b * 4 + hg]["out"]
        outp[b] = acc + bo
    return outp, res


def kernel(x, Wq, bq, Wk, bk, Wv, bv, Wo, bo):
    outp, _ = _run(x, Wq, bq, Wk, bk, Wv, bv, Wo, bo, trace=False)
    return outp


def kernel_traced(**inputs):
    return _run(trace=True, **inputs)
